# revision 38
# baseline (speedup 1.0000x reference)
"""BitLinear (4-bit activation quant + ternary weight) Trainium2 kernel.

Full computation:
    xq  = round(clip(x / max_abs(x, row) * 7)) * max_abs / 7      (per-row 4-bit quant)
    wq  = sign_thresholded(w) * mean_abs(w, row)                   (ternary weight)
    out = xq @ wq.T + bias

Strategy (8 NeuronCores, data-parallel over rows of x):
  - Shard x rows 8 ways; replicate weight.
  - x and weight ship to the device as f16 and the output returns as bf16
    (host upcasts to f32): DMA traffic drops from 36 MiB to 18.5 MiB per
    core. The f16 quant decisions + bf16 output quantization keep end-to-end
    rel err ~1.05e-2, inside the 2e-2 gate.
  - Matmul runs on exact small integers in fp8 (q in [-8,7], sign in
    {-1,0,1}) with DoubleRow perf mode (0.5 cycles/row). Rounding uses the
    +1.5*2^23 magic-number trick == round-half-even.
  - All weight prep (loads -> |w| row-sums -> global threshold -> ternary
    signs, with the A+B comparison sum accumulated in PSUM by the transpose
    matmuls and the -1 folded into the eviction bias) is emitted up front on
    otherwise-idle engines.
  - Steady-state engine balance per 128-row s-tile (~2.15us pace): DVE does
    absmax/scales and the column-scale multiply (2x-mode bf16 tensor_tensor);
    Pool does the magic multiply-add; ACT does both PSUM evictions (qt->fp8
    and matmul->bf16*rowscale); PE runs transposes + matmuls at full clock;
    SP issues all x loads and output stores.
"""

import os
import sys

os.environ.setdefault("MYCRO_LOCAL_CACHE", "1")

for _p in ("/opt/trn_rl_repo", "/root/.axon_site/_ro/trn_rl_repo"):
    if os.path.isdir(_p) and _p not in sys.path:
        sys.path.insert(0, _p)

import numpy as np

N_CORES = 8
S_SHARD = 4096
IN_F = 1024
OUT_F = 1024
P = 128
N_STILES = S_SHARD // P  # 32
N_KTILES = IN_F // P  # 8
N_OTILES = OUT_F // P  # 8
MM_N = 512
N_OHALF = OUT_F // MM_N  # 2
OUT_B = 4  # s-tiles per output store (2 MiB transfers)

MAGIC = 12582912.0
EPS = 1e-06

_prog_cache = {}


def _build_program(with_bias: bool):
    import concourse.bass as bass
    import concourse.mybir as mybir
    import concourse.tile as tile
    from concourse import bacc, bass_isa
    from concourse.masks import make_identity

    f32 = mybir.dt.float32
    f16 = mybir.dt.float16
    bf16 = mybir.dt.bfloat16
    f8 = mybir.dt.float8e4
    Alu = mybir.AluOpType
    Act = mybir.ActivationFunctionType

    nc = bacc.Bacc("TRN2", target_bir_lowering=False, debug=False)

    x_in = nc.dram_tensor("x_shard", [S_SHARD, IN_F], f16, kind="ExternalInput")
    w_in = nc.dram_tensor("weight", [OUT_F, IN_F], f16, kind="ExternalInput")
    if with_bias:
        b_in = nc.dram_tensor("bias", [OUT_F], f32, kind="ExternalInput")
    out_d = nc.dram_tensor("out", [S_SHARD, OUT_F], bf16, kind="ExternalOutput")

    WPREP_S = int(os.environ.get("KWPREP", "6"))
    SIGN_PER_S = int(os.environ.get("KSIGNPS", "4"))
    LEAD = int(os.environ.get("KLEAD", "2"))
    XBUFS = int(os.environ.get("KXBUFS", "12"))
    OBUFS = int(os.environ.get("KOBUFS", "6"))
    F2POOL = int(os.environ.get("KF2POOL", "0"))

    with tile.TileContext(nc) as tc:
        from contextlib import ExitStack as _ES

        _wstack = _ES()
        with (
            tc.tile_pool(name="singles", bufs=1) as singles,
            tc.tile_pool(name="wtmp", bufs=2) as wtmp,
            tc.tile_pool(name="signp", bufs=2) as signp,
            tc.tile_pool(name="xp", bufs=XBUFS) as xp,
            tc.tile_pool(name="tp", bufs=4) as tp,
            tc.tile_pool(name="fevp", bufs=4) as fevp,
            tc.tile_pool(name="qtp", bufs=LEAD + 3) as qtp,
            tc.tile_pool(name="outp", bufs=OBUFS) as outp,
            tc.tile_pool(name="stats", bufs=8) as stats,
            tc.tile_pool(name="ma7p", bufs=N_STILES + 1) as ma7p,
            tc.tile_pool(name="tpsum", bufs=2, space="PSUM") as tpsum,
            tc.tile_pool(name="mpsum", bufs=2, space="PSUM") as mpsum,
            tc.tile_pool(name="dramp", bufs=1, space="DRAM") as dramp,
        ):
            identity = singles.tile([P, P], bf16)
            make_identity(nc, identity)
            identity_f = singles.tile([P, P], f32)
            make_identity(nc, identity_f)

            magneg = singles.tile([P, 1], f32)
            nc.vector.memset(magneg, -MAGIC)
            magpos = singles.tile([P, 1], f32)
            nc.vector.memset(magpos, MAGIC)
            neg1 = singles.tile([P, 1], f32)
            nc.vector.memset(neg1, -1.0)

            signT8 = singles.tile([P, N_KTILES, OUT_F], f8)
            alpha_raw = singles.tile([P, N_OTILES], f32)

            wpool = _wstack.enter_context(tc.tile_pool(name="wpool", bufs=8))
            w_tiles = []
            for j in range(N_OTILES):
                w_t = wpool.tile([P, IN_F], f16, tag="w")
                w_tiles.append(w_t)

            def emit_wload(js):
                for j in js:
                    if j < N_OTILES:
                        nc.sync.dma_start(
                            out=w_tiles[j], in_=w_in[j * P : (j + 1) * P, :]
                        )

            def emit_wabs(j):
                if j % 2 == 0:
                    nc.vector.tensor_reduce(
                        out=alpha_raw[:, j : j + 1],
                        in_=w_tiles[j],
                        axis=mybir.AxisListType.X,
                        op=Alu.add,
                        apply_absolute_value=True,
                    )
                else:
                    dump = wtmp.tile([P, IN_F], f16, tag="dump")
                    nc.scalar.activation(
                        out=dump,
                        in_=w_tiles[j],
                        func=Act.Abs,
                        accum_out=alpha_raw[:, j : j + 1],
                    )

            x_pairs = {}

            def emit_quant(s):
                if s % 2 == 0:
                    x2 = xp.tile([P, 2, IN_F], f16, tag="x")
                    if s == 0:
                        for g in range(2):
                            nc.sync.dma_start(
                                out=x2[:, g, :],
                                in_=x_in[(s + g) * P : (s + g + 1) * P, :],
                            )
                    else:
                        nc.sync.dma_start(
                            out=x2,
                            in_=x_in[s * P : (s + 2) * P, :].rearrange(
                                "(two p) f -> p two f", p=P
                            ),
                        )
                    x_pairs[s] = x2
                    x_t = x2[:, 0, :]
                else:
                    x_t = x_pairs.pop(s - 1)[:, 1, :]
                ma = stats.tile([P, 1], f32, tag="ma")
                nc.vector.tensor_reduce(
                    out=ma,
                    in_=x_t,
                    axis=mybir.AxisListType.X,
                    op=Alu.max,
                    apply_absolute_value=True,
                )
                ma7 = ma7p.tile([P, 1], f32, tag="ma7")
                nc.vector.tensor_scalar(
                    out=ma7,
                    in0=ma,
                    scalar1=float(1.0 / 7.0),
                    scalar2=float(EPS / 7.0),
                    op0=Alu.mult,
                    op1=Alu.max,
                )
                inv = stats.tile([P, 1], f32, tag="inv")
                nc.vector.reciprocal(out=inv, in_=ma7)
                # t = x*inv + MAGIC (f32; fraction now rounded half-to-even)
                t_t = tp.tile([P, IN_F], f32, tag="t")
                b_eng = nc.gpsimd
                b_eng.tensor_scalar(
                    out=t_t,
                    in0=x_t,
                    scalar1=inv,
                    scalar2=MAGIC,
                    op0=Alu.mult,
                    op1=Alu.add,
                )
                qt_ps = tpsum.tile([P, IN_F], f32, tag="tps")
                for k in range(N_KTILES):
                    nc.tensor.transpose(
                        qt_ps[:, k * P : (k + 1) * P],
                        t_t[:, k * P : (k + 1) * P],
                        identity_f,
                    )
                qt_sb = qtp.tile([P, N_KTILES, P], f8, tag="qt")
                nc.scalar.activation(
                    out=qt_sb.rearrange("p k c -> p (k c)"),
                    in_=qt_ps,
                    func=Act.Identity,
                    bias=magneg,
                    scale=1.0,
                )
                return ma7, qt_sb

            out_blocks = {}

            def emit_matmul(s, ma7, qt_sb):
                sb = (s // OUT_B) * OUT_B
                if s % OUT_B == 0:
                    ob = outp.tile([P, OUT_B, OUT_F], bf16, tag="o")
                    out_blocks[sb] = ob
                else:
                    ob = out_blocks[sb]
                out_sb = ob[:, s % OUT_B, :]
                ps = mpsum.tile([P, OUT_F], f32, tag="mm")
                for h in range(N_OHALF):
                    for t in range(N_KTILES // 2):
                        nc.tensor.matmul(
                            ps[:, h * MM_N : (h + 1) * MM_N],
                            lhsT=qt_sb[:, 2 * t : 2 * t + 2, :],
                            rhs=signT8[
                                :, 2 * t : 2 * t + 2, h * MM_N : (h + 1) * MM_N
                            ],
                            start=(t == 0),
                            stop=(t == N_KTILES // 2 - 1),
                            perf_mode=mybir.MatmulPerfMode.DoubleRow,
                        )
                # out = (S * rowscale) * colscale: ACT evicts PSUM with the
                # per-row scale (GPSIMD can't read PSUM), then the per-column
                # multiply runs on DVE in 2x mode (all-bf16 operands).
                fev = fevp.tile([P, OUT_F], bf16, tag="fev")
                nc.scalar.activation(
                    out=fev, in_=ps, func=Act.Identity, scale=ma7
                )
                f2_eng = nc.gpsimd if (F2POOL and s % F2POOL == 0) else nc.vector
                f2_eng.tensor_tensor(
                    out=out_sb, in0=fev, in1=colb, op=Alu.mult
                )
                if with_bias:
                    nc.gpsimd.tensor_tensor(
                        out=out_sb, in0=out_sb, in1=biasb, op=Alu.add
                    )
                if s % 2 == 1 and sb >= N_STILES - 2 * OUT_B and s % OUT_B == 1:
                    nc.sync.dma_start(
                        out=out_d[sb * P : (sb + 2) * P, :].rearrange(
                            "(b p) f -> p b f", p=P
                        ),
                        in_=out_blocks[sb][:, : OUT_B - 2, :],
                    )
                elif s % OUT_B == OUT_B - 1:
                    if sb >= N_STILES - 2 * OUT_B:
                        nc.sync.dma_start(
                            out=out_d[(sb + 2) * P : (s + 1) * P, :].rearrange(
                                "(b p) f -> p b f", p=P
                            ),
                            in_=out_blocks.pop(sb)[:, OUT_B - 2 :, :],
                        )
                    else:
                        nc.sync.dma_start(
                            out=out_d[sb * P : (s + 1) * P, :].rearrange(
                                "(b p) f -> p b f", p=P
                            ),
                            in_=out_blocks.pop(sb),
                        )

            def emit_wprep_head():
                # global threshold = 0.05 * mean(|w|)
                g0 = stats.tile([P, 1], f32, tag="g0")
                nc.vector.tensor_reduce(
                    out=g0, in_=alpha_raw, axis=mybir.AxisListType.X, op=Alu.add
                )
                g1 = stats.tile([P, 1], f32, tag="g1")
                nc.gpsimd.partition_all_reduce(
                    out_ap=g1, in_ap=g0, channels=P, reduce_op=bass_isa.ReduceOp.add
                )
                nc.vector.tensor_scalar(
                    out=thr,
                    in0=g1,
                    scalar1=float(0.05 / (OUT_F * IN_F)),
                    scalar2=None,
                    op0=Alu.mult,
                )
                nc.vector.tensor_scalar(
                    out=nthr, in0=thr, scalar1=-1.0, scalar2=None, op0=Alu.mult
                )
                nc.vector.tensor_scalar(
                    out=alpha_sb,
                    in0=alpha_raw,
                    scalar1=float(1.0 / IN_F),
                    scalar2=None,
                    op0=Alu.mult,
                )
                # column scale alpha broadcast to all partitions via DRAM bounce
                nc.gpsimd.dma_start(
                    out=alpha_dram.rearrange("j p -> p j"), in_=alpha_sb
                )
                alpha_flat = alpha_dram.rearrange("j p -> (j p)")
                bcast_src = bass.AP(
                    tensor=alpha_flat.tensor,
                    offset=alpha_flat.offset,
                    ap=[[0, P]] + list(alpha_flat.ap),
                )
                nc.gpsimd.dma_start(out=colb, in_=bcast_src)
                if with_bias:
                    bias_src = bass.AP(
                        tensor=b_in.tensor
                        if hasattr(b_in, "tensor")
                        else b_in[:].tensor,
                        offset=b_in[:].offset,
                        ap=[[0, P]] + list(b_in[:].ap),
                    )
                    nc.sync.dma_start(out=biasb, in_=bias_src)

            def emit_sign(j):
                # ternary sign: sign = (w >= thr) + (w > -thr) - 1.
                # The two comparisons are cheap 4x-mode DVE tensor_scalars;
                # the add happens for free in PSUM (accumulating transposes)
                # and the -1 rides the eviction bias.
                a_cmp = wtmp.tile([P, IN_F], bf16, tag="tmp")
                nc.vector.tensor_scalar(
                    out=a_cmp,
                    in0=w_tiles[j],
                    scalar1=nthr,
                    scalar2=None,
                    op0=Alu.is_gt,
                )
                b_cmp = signp.tile([P, IN_F], bf16, tag="sgn")
                nc.vector.tensor_scalar(
                    out=b_cmp,
                    in0=w_tiles[j],
                    scalar1=thr,
                    scalar2=None,
                    op0=Alu.is_ge,
                )
                pool_j = tpsum if j % 2 == 0 else mpsum
                ps = pool_j.tile([P, IN_F], f32, tag="tps" if j % 2 == 0 else "mm")
                for k in range(N_KTILES):
                    nc.tensor.matmul(
                        ps[:, k * P : (k + 1) * P],
                        lhsT=a_cmp[:, k * P : (k + 1) * P],
                        rhs=identity,
                        start=True,
                        stop=False,
                    )
                    nc.tensor.matmul(
                        ps[:, k * P : (k + 1) * P],
                        lhsT=b_cmp[:, k * P : (k + 1) * P],
                        rhs=identity,
                        start=False,
                        stop=True,
                    )
                nc.scalar.activation(
                    out=signT8[:, :, j * P : (j + 1) * P],
                    in_=ps.rearrange("p (k c) -> p k c", k=N_KTILES),
                    func=Act.Identity,
                    bias=neg1,
                )

            thr = singles.tile([P, 1], f32)
            nthr = singles.tile([P, 1], f32)
            alpha_sb = singles.tile([P, N_OTILES], bf16)
            alpha_dram = dramp.tile([N_OTILES, P], bf16)
            colb = singles.tile([P, OUT_F], bf16)
            biasb = None
            biasf = None
            if with_bias:
                biasf = singles.tile([P, OUT_F], f32, tag="biasf")
                biasb = singles.tile([P, OUT_F], bf16, tag="biasb")

            # All weight prep runs up front: w loads lead the DMA queue,
            # row-sums/threshold/signs occupy the otherwise-idle engines
            # while x streams in behind the weights.
            emit_wload(range(N_OTILES))
            for j in range(N_OTILES):
                emit_wabs(j)
            emit_wprep_head()
            for j in range(N_OTILES):
                emit_sign(j)
            w_tiles.clear()
            _wstack.close()
            prologue = []
            for s in range(N_STILES):
                prologue.append(emit_quant(s))
                if s >= LEAD:
                    emit_matmul(s - LEAD, *prologue[s - LEAD])
            for s in range(max(0, N_STILES - LEAD), N_STILES):
                emit_matmul(s, *prologue[s])

    nc.compile()
    return nc


def _get_program(with_bias: bool):
    key = bool(with_bias)
    if key not in _prog_cache:
        _prog_cache[key] = _build_program(key)
    return _prog_cache[key]


def kernel(x: np.ndarray, weight: np.ndarray, bias: np.ndarray) -> np.ndarray:
    from concourse.bass_utils import run_bass_kernel_spmd

    B, S, in_f = x.shape
    out_f = weight.shape[0]
    assert in_f == IN_F and out_f == OUT_F and B * S == N_CORES * S_SHARD

    xf = np.ascontiguousarray(
        x.astype(np.float16, copy=False).reshape(-1, IN_F)
    )
    w = np.ascontiguousarray(weight.astype(np.float16, copy=False))
    b = np.ascontiguousarray(bias.astype(np.float32, copy=False))

    with_bias = bool(np.any(b != 0.0))
    nc = _get_program(with_bias)

    in_maps = []
    for c in range(N_CORES):
        m = {
            "x_shard": xf[c * S_SHARD : (c + 1) * S_SHARD],
            "weight": w,
        }
        if with_bias:
            m["bias"] = b
        in_maps.append(m)

    res = run_bass_kernel_spmd(nc, in_maps, core_ids=list(range(N_CORES)))
    out = np.concatenate(
        [np.asarray(res.results[c]["out"]) for c in range(N_CORES)], axis=0
    )
    return out.reshape(B, S, OUT_F).astype(np.float32)


# revision 39
# speedup vs baseline: 1.0037x; 1.0037x over previous
"""BitLinear (4-bit activation quant + ternary weight) Trainium2 kernel.

Full computation:
    xq  = round(clip(x / max_abs(x, row) * 7)) * max_abs / 7      (per-row 4-bit quant)
    wq  = sign_thresholded(w) * mean_abs(w, row)                   (ternary weight)
    out = xq @ wq.T + bias

Strategy (8 NeuronCores, data-parallel over rows of x):
  - Shard x rows 8 ways; replicate weight.
  - x and weight ship to the device as f16 and the output returns as bf16
    (host upcasts to f32): DMA traffic drops from 36 MiB to 18.5 MiB per
    core. The f16 quant decisions + bf16 output quantization keep end-to-end
    rel err ~1.05e-2, inside the 2e-2 gate.
  - Matmul runs on exact small integers in fp8 (q in [-8,7], sign in
    {-1,0,1}) with DoubleRow perf mode (0.5 cycles/row). Rounding uses the
    +1.5*2^23 magic-number trick == round-half-even.
  - All weight prep (loads -> |w| row-sums -> global threshold -> ternary
    signs, with the A+B comparison sum accumulated in PSUM by the transpose
    matmuls and the -1 folded into the eviction bias) is emitted up front on
    otherwise-idle engines.
  - Steady-state engine balance per 128-row s-tile (~2.15us pace): DVE does
    absmax/scales and the column-scale multiply (2x-mode bf16 tensor_tensor);
    Pool does the magic multiply-add; ACT does both PSUM evictions (qt->fp8
    and matmul->bf16*rowscale); PE runs transposes + matmuls at full clock;
    SP issues all x loads and output stores.
"""

import os
import sys

os.environ.setdefault("MYCRO_LOCAL_CACHE", "1")

for _p in ("/opt/trn_rl_repo", "/root/.axon_site/_ro/trn_rl_repo"):
    if os.path.isdir(_p) and _p not in sys.path:
        sys.path.insert(0, _p)

import numpy as np

N_CORES = 8
S_SHARD = 4096
IN_F = 1024
OUT_F = 1024
P = 128
N_STILES = S_SHARD // P  # 32
N_KTILES = IN_F // P  # 8
N_OTILES = OUT_F // P  # 8
MM_N = 512
N_OHALF = OUT_F // MM_N  # 2
OUT_B = 4  # s-tiles per output store (2 MiB transfers)

MAGIC = 12582912.0
EPS = 1e-06

_prog_cache = {}


def _build_program(with_bias: bool):
    import concourse.bass as bass
    import concourse.mybir as mybir
    import concourse.tile as tile
    from concourse import bacc, bass_isa
    from concourse.masks import make_identity

    f32 = mybir.dt.float32
    f16 = mybir.dt.float16
    bf16 = mybir.dt.bfloat16
    f8 = mybir.dt.float8e4
    Alu = mybir.AluOpType
    Act = mybir.ActivationFunctionType

    nc = bacc.Bacc("TRN2", target_bir_lowering=False, debug=False)

    x_in = nc.dram_tensor("x_shard", [S_SHARD, IN_F], f16, kind="ExternalInput")
    w_in = nc.dram_tensor("weight", [OUT_F, IN_F], f16, kind="ExternalInput")
    if with_bias:
        b_in = nc.dram_tensor("bias", [OUT_F], f32, kind="ExternalInput")
    out_d = nc.dram_tensor("out", [S_SHARD, OUT_F], bf16, kind="ExternalOutput")

    WPREP_S = int(os.environ.get("KWPREP", "6"))
    SIGN_PER_S = int(os.environ.get("KSIGNPS", "4"))
    LEAD = int(os.environ.get("KLEAD", "2"))
    XBUFS = int(os.environ.get("KXBUFS", "12"))
    OBUFS = int(os.environ.get("KOBUFS", "6"))
    F2POOL = int(os.environ.get("KF2POOL", "0"))

    with tile.TileContext(nc) as tc:
        from contextlib import ExitStack as _ES

        _wstack = _ES()
        with (
            tc.tile_pool(name="singles", bufs=1) as singles,
            tc.tile_pool(name="wtmp", bufs=2) as wtmp,
            tc.tile_pool(name="signp", bufs=2) as signp,
            tc.tile_pool(name="xp", bufs=XBUFS) as xp,
            tc.tile_pool(name="tp", bufs=4) as tp,
            tc.tile_pool(name="fevp", bufs=4) as fevp,
            tc.tile_pool(name="qtp", bufs=LEAD + 3) as qtp,
            tc.tile_pool(name="outp", bufs=OBUFS) as outp,
            tc.tile_pool(name="stats", bufs=8) as stats,
            tc.tile_pool(name="ma7p", bufs=N_STILES + 1) as ma7p,
            tc.tile_pool(name="tpsum", bufs=2, space="PSUM") as tpsum,
            tc.tile_pool(name="mpsum", bufs=2, space="PSUM") as mpsum,
            tc.tile_pool(name="dramp", bufs=1, space="DRAM") as dramp,
        ):
            identity = singles.tile([P, P], bf16)
            make_identity(nc, identity)
            identity_f = singles.tile([P, P], f32)
            make_identity(nc, identity_f)

            magneg = singles.tile([P, 1], f32)
            nc.vector.memset(magneg, -MAGIC)
            magpos = singles.tile([P, 1], f32)
            nc.vector.memset(magpos, MAGIC)
            neg1 = singles.tile([P, 1], f32)
            nc.vector.memset(neg1, -1.0)

            signT8 = singles.tile([P, N_KTILES, OUT_F], f8)
            alpha_raw = singles.tile([P, N_OTILES], f32)

            wpool = _wstack.enter_context(tc.tile_pool(name="wpool", bufs=8))
            w_tiles = []
            for j in range(N_OTILES):
                w_t = wpool.tile([P, IN_F], f16, tag="w")
                w_tiles.append(w_t)

            def emit_wload(js):
                for j in js:
                    if j < N_OTILES:
                        nc.sync.dma_start(
                            out=w_tiles[j], in_=w_in[j * P : (j + 1) * P, :]
                        )

            def emit_wabs(j):
                if j % 2 == 0:
                    nc.vector.tensor_reduce(
                        out=alpha_raw[:, j : j + 1],
                        in_=w_tiles[j],
                        axis=mybir.AxisListType.X,
                        op=Alu.add,
                        apply_absolute_value=True,
                    )
                else:
                    dump = wtmp.tile([P, IN_F], f16, tag="dump")
                    nc.scalar.activation(
                        out=dump,
                        in_=w_tiles[j],
                        func=Act.Abs,
                        accum_out=alpha_raw[:, j : j + 1],
                    )

            x_pairs = {}

            def emit_quant(s):
                if s % 2 == 0:
                    x2 = xp.tile([P, 2, IN_F], f16, tag="x")
                    if s == 0:
                        for g in range(2):
                            nc.sync.dma_start(
                                out=x2[:, g, :],
                                in_=x_in[(s + g) * P : (s + g + 1) * P, :],
                            )
                    else:
                        nc.sync.dma_start(
                            out=x2,
                            in_=x_in[s * P : (s + 2) * P, :].rearrange(
                                "(two p) f -> p two f", p=P
                            ),
                        )
                    x_pairs[s] = x2
                    x_t = x2[:, 0, :]
                else:
                    x_t = x_pairs.pop(s - 1)[:, 1, :]
                ma = stats.tile([P, 1], f32, tag="ma")
                nc.vector.tensor_reduce(
                    out=ma,
                    in_=x_t,
                    axis=mybir.AxisListType.X,
                    op=Alu.max,
                    apply_absolute_value=True,
                )
                ma7 = ma7p.tile([P, 1], f32, tag="ma7")
                nc.vector.tensor_scalar(
                    out=ma7,
                    in0=ma,
                    scalar1=float(1.0 / 7.0),
                    scalar2=float(EPS / 7.0),
                    op0=Alu.mult,
                    op1=Alu.max,
                )
                inv = stats.tile([P, 1], f32, tag="inv")
                nc.vector.reciprocal(out=inv, in_=ma7)
                # t = x*inv + MAGIC (f32; fraction now rounded half-to-even)
                t_t = tp.tile([P, IN_F], f32, tag="t")
                b_eng = nc.gpsimd
                b_eng.tensor_scalar(
                    out=t_t,
                    in0=x_t,
                    scalar1=inv,
                    scalar2=MAGIC,
                    op0=Alu.mult,
                    op1=Alu.add,
                )
                qt_ps = tpsum.tile([P, IN_F], f32, tag="tps")
                for k in range(N_KTILES):
                    nc.tensor.transpose(
                        qt_ps[:, k * P : (k + 1) * P],
                        t_t[:, k * P : (k + 1) * P],
                        identity_f,
                    )
                qt_sb = qtp.tile([P, N_KTILES, P], f8, tag="qt")
                nc.scalar.activation(
                    out=qt_sb.rearrange("p k c -> p (k c)"),
                    in_=qt_ps,
                    func=Act.Identity,
                    bias=magneg,
                    scale=1.0,
                )
                return ma7, qt_sb

            out_blocks = {}

            def emit_matmul(s, ma7, qt_sb):
                sb = (s // OUT_B) * OUT_B
                if s % OUT_B == 0:
                    ob = outp.tile([P, OUT_B, OUT_F], bf16, tag="o")
                    out_blocks[sb] = ob
                else:
                    ob = out_blocks[sb]
                out_sb = ob[:, s % OUT_B, :]
                ps = mpsum.tile([P, OUT_F], f32, tag="mm")
                for h in range(N_OHALF):
                    for t in range(N_KTILES // 2):
                        nc.tensor.matmul(
                            ps[:, h * MM_N : (h + 1) * MM_N],
                            lhsT=qt_sb[:, 2 * t : 2 * t + 2, :],
                            rhs=signT8[
                                :, 2 * t : 2 * t + 2, h * MM_N : (h + 1) * MM_N
                            ],
                            start=(t == 0),
                            stop=(t == N_KTILES // 2 - 1),
                            perf_mode=mybir.MatmulPerfMode.DoubleRow,
                        )
                # out = (S * rowscale) * colscale: ACT evicts PSUM with the
                # per-row scale (GPSIMD can't read PSUM), then the per-column
                # multiply runs on DVE in 2x mode (all-bf16 operands).
                fev = fevp.tile([P, OUT_F], bf16, tag="fev")
                nc.scalar.activation(
                    out=fev, in_=ps, func=Act.Identity, scale=ma7
                )
                f2_eng = nc.gpsimd if (F2POOL and s % F2POOL == 0) else nc.vector
                f2_eng.tensor_tensor(
                    out=out_sb, in0=fev, in1=colb, op=Alu.mult
                )
                if with_bias:
                    nc.gpsimd.tensor_tensor(
                        out=out_sb, in0=out_sb, in1=biasb, op=Alu.add
                    )
                if s % 2 == 1 and sb >= N_STILES - 2 * OUT_B and s % OUT_B == 1:
                    nc.sync.dma_start(
                        out=out_d[sb * P : (sb + 2) * P, :].rearrange(
                            "(b p) f -> p b f", p=P
                        ),
                        in_=out_blocks[sb][:, : OUT_B - 2, :],
                    )
                elif s % OUT_B == OUT_B - 1:
                    if sb >= N_STILES - 2 * OUT_B:
                        nc.sync.dma_start(
                            out=out_d[(sb + 2) * P : (s + 1) * P, :].rearrange(
                                "(b p) f -> p b f", p=P
                            ),
                            in_=out_blocks.pop(sb)[:, OUT_B - 2 :, :],
                        )
                    else:
                        nc.sync.dma_start(
                            out=out_d[sb * P : (s + 1) * P, :].rearrange(
                                "(b p) f -> p b f", p=P
                            ),
                            in_=out_blocks.pop(sb),
                        )

            def emit_wprep_head():
                # global threshold = 0.05 * mean(|w|)
                g0 = stats.tile([P, 1], f32, tag="g0")
                nc.vector.tensor_reduce(
                    out=g0, in_=alpha_raw, axis=mybir.AxisListType.X, op=Alu.add
                )
                g1 = stats.tile([P, 1], f32, tag="g1")
                nc.gpsimd.partition_all_reduce(
                    out_ap=g1, in_ap=g0, channels=P, reduce_op=bass_isa.ReduceOp.add
                )
                nc.vector.tensor_scalar(
                    out=thr,
                    in0=g1,
                    scalar1=float(0.05 / (OUT_F * IN_F)),
                    scalar2=None,
                    op0=Alu.mult,
                )
                nc.vector.tensor_scalar(
                    out=nthr, in0=thr, scalar1=-1.0, scalar2=None, op0=Alu.mult
                )
                nc.vector.tensor_scalar(
                    out=alpha_sb,
                    in0=alpha_raw,
                    scalar1=float(1.0 / IN_F),
                    scalar2=None,
                    op0=Alu.mult,
                )
                # column scale alpha broadcast to all partitions via DRAM bounce
                nc.gpsimd.dma_start(
                    out=alpha_dram.rearrange("j p -> p j"), in_=alpha_sb
                )
                alpha_flat = alpha_dram.rearrange("j p -> (j p)")
                bcast_src = bass.AP(
                    tensor=alpha_flat.tensor,
                    offset=alpha_flat.offset,
                    ap=[[0, P]] + list(alpha_flat.ap),
                )
                nc.gpsimd.dma_start(out=colb, in_=bcast_src)
                if with_bias:
                    bias_src = bass.AP(
                        tensor=b_in.tensor
                        if hasattr(b_in, "tensor")
                        else b_in[:].tensor,
                        offset=b_in[:].offset,
                        ap=[[0, P]] + list(b_in[:].ap),
                    )
                    nc.sync.dma_start(out=biasb, in_=bias_src)

            def emit_sign(j):
                # ternary sign: sign = (w >= thr) + (w > -thr) - 1.
                # The two comparisons are cheap 4x-mode DVE tensor_scalars;
                # the add happens for free in PSUM (accumulating transposes)
                # and the -1 rides the eviction bias.
                a_cmp = wtmp.tile([P, IN_F], bf16, tag="tmp")
                nc.vector.tensor_scalar(
                    out=a_cmp,
                    in0=w_tiles[j],
                    scalar1=nthr,
                    scalar2=None,
                    op0=Alu.is_gt,
                )
                b_cmp = signp.tile([P, IN_F], bf16, tag="sgn")
                nc.vector.tensor_scalar(
                    out=b_cmp,
                    in0=w_tiles[j],
                    scalar1=thr,
                    scalar2=None,
                    op0=Alu.is_ge,
                )
                pool_j = tpsum if j % 2 == 0 else mpsum
                ps = pool_j.tile([P, IN_F], f32, tag="tps" if j % 2 == 0 else "mm")
                for k in range(N_KTILES):
                    nc.tensor.matmul(
                        ps[:, k * P : (k + 1) * P],
                        lhsT=a_cmp[:, k * P : (k + 1) * P],
                        rhs=identity,
                        start=True,
                        stop=False,
                    )
                    nc.tensor.matmul(
                        ps[:, k * P : (k + 1) * P],
                        lhsT=b_cmp[:, k * P : (k + 1) * P],
                        rhs=identity,
                        start=False,
                        stop=True,
                    )
                if j < 6:
                    nc.scalar.activation(
                        out=signT8[:, :, j * P : (j + 1) * P],
                        in_=ps.rearrange("p (k c) -> p k c", k=N_KTILES),
                        func=Act.Identity,
                        bias=neg1,
                    )
                else:
                    nc.vector.tensor_scalar(
                        out=signT8[:, :, j * P : (j + 1) * P],
                        in0=ps.rearrange("p (k c) -> p k c", k=N_KTILES),
                        scalar1=-1.0,
                        scalar2=None,
                        op0=Alu.add,
                    )

            thr = singles.tile([P, 1], f32)
            nthr = singles.tile([P, 1], f32)
            alpha_sb = singles.tile([P, N_OTILES], bf16)
            alpha_dram = dramp.tile([N_OTILES, P], bf16)
            colb = singles.tile([P, OUT_F], bf16)
            biasb = None
            biasf = None
            if with_bias:
                biasf = singles.tile([P, OUT_F], f32, tag="biasf")
                biasb = singles.tile([P, OUT_F], bf16, tag="biasb")

            # All weight prep runs up front: w loads lead the DMA queue,
            # row-sums/threshold/signs occupy the otherwise-idle engines
            # while x streams in behind the weights.
            emit_wload(range(N_OTILES))
            for j in range(N_OTILES):
                emit_wabs(j)
            emit_wprep_head()
            for j in range(N_OTILES):
                emit_sign(j)
            w_tiles.clear()
            _wstack.close()
            prologue = []
            for s in range(N_STILES):
                prologue.append(emit_quant(s))
                if s >= LEAD:
                    emit_matmul(s - LEAD, *prologue[s - LEAD])
            for s in range(max(0, N_STILES - LEAD), N_STILES):
                emit_matmul(s, *prologue[s])

    nc.compile()
    return nc


def _get_program(with_bias: bool):
    key = bool(with_bias)
    if key not in _prog_cache:
        _prog_cache[key] = _build_program(key)
    return _prog_cache[key]


def kernel(x: np.ndarray, weight: np.ndarray, bias: np.ndarray) -> np.ndarray:
    from concourse.bass_utils import run_bass_kernel_spmd

    B, S, in_f = x.shape
    out_f = weight.shape[0]
    assert in_f == IN_F and out_f == OUT_F and B * S == N_CORES * S_SHARD

    xf = np.ascontiguousarray(
        x.astype(np.float16, copy=False).reshape(-1, IN_F)
    )
    w = np.ascontiguousarray(weight.astype(np.float16, copy=False))
    b = np.ascontiguousarray(bias.astype(np.float32, copy=False))

    with_bias = bool(np.any(b != 0.0))
    nc = _get_program(with_bias)

    in_maps = []
    for c in range(N_CORES):
        m = {
            "x_shard": xf[c * S_SHARD : (c + 1) * S_SHARD],
            "weight": w,
        }
        if with_bias:
            m["bias"] = b
        in_maps.append(m)

    res = run_bass_kernel_spmd(nc, in_maps, core_ids=list(range(N_CORES)))
    out = np.concatenate(
        [np.asarray(res.results[c]["out"]) for c in range(N_CORES)], axis=0
    )
    return out.reshape(B, S, OUT_F).astype(np.float32)
